# revision 1
# baseline (speedup 1.0000x reference)
"""DigitCaps (capsule routing) Trainium2 Bass kernel.

u [512, 1152, 8] f32, W [1, 1152, 10, 16, 8] f32 -> v [512, 10, 16] f32
(3 dynamic-routing iterations, softmax over 10 classes).

Pure data-parallel: batch 64 per core x 8 cores; everything on-chip;
u_hat (377MB) is never materialized. Per routing iteration:
  T[b,i,c,k] = sum_d W[i,c,d,k] v[b,c,d]     PE (lhsT = W rows (c2,d16),
                                              rhs = block-diag v^T;
                                              output i-major)
  L[b,i,c]  += sum_k u[b,i,k] T[b,i,c,k]     DVE mul + tree adds (bf16)
  cexp       = exp(L) (ACT); den/rec via DVE adds+recip (all i-major)
  x_c        = cexp_c * (u * recT)           DVE (per class)
  s[b,c,:]   = sum_{ik} W x_c                PE (72 accumulating matmuls)
  v          = squash(s)
Everything softmax/logit-related lives i-major, so no per-iteration
layout transposes are needed anywhere.

Layouts (per core, B=64):
  i: block g = i//128 (9 blocks), partition r = i%128
  class c = 2p+ch, pass p in [0,5), parity ch in {0,1}
  logits/exp: [r, p, (g, ch, b)]
"""

import numpy as np

N_CORES = 8
B_PER = 64
I_CAPS = 1152
K_DIM = 8
C_CLS = 10
D_DIM = 16
NG = I_CAPS // 128  # 9
EPS = 1e-8

_CACHE = {}


def _build():
    import concourse.bass as bass
    import concourse.mybir as mybir
    from concourse import tile, bacc

    f32 = mybir.dt.float32
    bf16 = mybir.dt.bfloat16
    AF = mybir.ActivationFunctionType
    OP = mybir.AluOpType

    nc = bacc.Bacc()
    uTk_in = nc.dram_tensor(
        "uTk_h", [128, K_DIM, NG, B_PER], bf16, kind="ExternalInput"
    )
    wsk_in = nc.dram_tensor(
        "wsk_h", [128, K_DIM, NG, C_CLS, D_DIM], bf16, kind="ExternalInput"
    )
    wt_in = nc.dram_tensor("wt_h", [128, K_DIM, I_CAPS], bf16, kind="ExternalInput")
    wtb_in = nc.dram_tensor("wtb_h", [128, K_DIM, I_CAPS], bf16, kind="ExternalInput")
    eye128 = nc.dram_tensor("eye128", [128, 128], f32, kind="ExternalInput")
    v_out = nc.dram_tensor("v", [B_PER, C_CLS, D_DIM], f32, kind="ExternalOutput")

    with tile.TileContext(nc) as tc:
        perm = tc.alloc_tile_pool(name="perm", bufs=1)
        Wsk = perm.tile([128, K_DIM, NG, C_CLS, D_DIM], bf16)  # [r,(k,g,c,d)]
        WT = perm.tile([128, K_DIM, I_CAPS], bf16)   # rows 16c+d (classes 0-7)
        WTB = perm.tile([128, K_DIM, I_CAPS], bf16)  # rows 16(c-2)+d; 96:128 used
        uTk = perm.tile([128, K_DIM, NG, B_PER], bf16)      # u[b, 128g+r, k]
        L = perm.tile([128, 5, NG, 2, B_PER], bf16, name="Lt")    # logits i-major
        cE = perm.tile([128, 5, NG, 2, B_PER], bf16, name="cEt")  # exp(L)
        recT = perm.tile([128, NG, B_PER], bf16, name="recTt")    # 1/den i-major
        vT = perm.tile([128, 128], bf16)             # block-diag v^T classes 0-7
        vT4 = perm.tile([128, 128], bf16)            # rows 96:128: classes 8,9
        v_sb = perm.tile([64, C_CLS, D_DIM], f32, name="vsbt")
        s_sb = perm.tile([64, C_CLS, D_DIM], f32, name="ssbt")
        eye_sb = perm.tile([128, 128], f32)
        in2 = perm.tile([128, 128], f32)
        in2b = perm.tile([128, 128], f32)
        sq = perm.tile([64, C_CLS, D_DIM], f32)
        n2 = perm.tile([64, C_CLS], f32)
        t1 = perm.tile([64, C_CLS], f32)
        r1 = perm.tile([64, C_CLS], f32)
        f1 = perm.tile([64, C_CLS], f32)
        nrm = perm.tile([64, C_CLS], f32)
        nrm2 = perm.tile([64, C_CLS], f32)
        r2 = perm.tile([64, C_CLS], f32)
        fac = perm.tile([64, C_CLS], f32)

        psS = tc.alloc_tile_pool(name="psS", bufs=2, space="PSUM")
        psT = tc.alloc_tile_pool(name="psT", bufs=2, space="PSUM")

        # ---------------- setup: inputs arrive pre-arranged ----
        # order: s0's inputs first, then WT/WTB in k-halves so the first
        # T-matmul passes can start while the rest streams in
        nc.sync.dma_start(uTk[:, 0:4], uTk_in[:, 0:4])
        nc.sync.dma_start(Wsk[:, 0:4], wsk_in[:, 0:4])
        nc.sync.dma_start(uTk[:, 4:8], uTk_in[:, 4:8])
        nc.sync.dma_start(Wsk[:, 4:8], wsk_in[:, 4:8])
        nc.sync.dma_start(eye_sb[:], eye128[:])
        nc.sync.dma_start(WT[:, 0:4, :], wt_in[:, 0:4, :])
        nc.sync.dma_start(WTB[:, 0:4, :], wtb_in[:, 0:4, :])
        nc.sync.dma_start(WT[:, 4:8, :], wt_in[:, 4:8, :])
        nc.sync.dma_start(WTB[:, 4:8, :], wtb_in[:, 4:8, :])

        nc.vector.memset(in2[:], 0.0)
        nc.vector.memset(in2b[:], 0.0)
        nc.gpsimd.memset(L[:], 0.0)

        itp = tc.alloc_tile_pool(name="itp", bufs=2)
        smp = tc.alloc_tile_pool(name="smp", bufs=3)

        def s_phase_s0():
            ps = psS.tile([64, C_CLS * D_DIM], f32, tag="ps_s")
            n = 0
            for k in range(K_DIM):
                for g in range(NG):
                    nc.tensor.matmul(
                        ps[:],
                        uTk[:, k, g, :],
                        Wsk[:, k, g, :, :].rearrange("r c d -> r (c d)"),
                        start=(n == 0),
                        stop=(n == K_DIM * NG - 1),
                    )
                    n += 1
            nc.scalar.activation(
                s_sb[:].rearrange("b c d -> b (c d)"), ps[:], AF.Copy, scale=0.1
            )

        def squash():
            nc.scalar.square(sq[:], s_sb[:])
            nc.vector.reduce_sum(n2[:], sq[:], axis=mybir.AxisListType.X)
            nc.scalar.add(t1[:], n2[:], 1.0)
            nc.vector.reciprocal(r1[:], t1[:])
            nc.vector.tensor_mul(f1[:], n2[:], r1[:])
            nc.scalar.sqrt(nrm[:], n2[:])
            nc.vector.tensor_scalar_add(nrm2[:], nrm[:], EPS)
            nc.vector.reciprocal(r2[:], nrm2[:])
            nc.vector.tensor_mul(fac[:], f1[:], r2[:])
            for c in range(C_CLS):
                nc.vector.tensor_scalar_mul(
                    v_sb[:, c, :], s_sb[:, c, :], fac[:, c : c + 1]
                )

        def build_vT():
            # in2[64ch+b, 16c+d] = v[b,c,d] for c%2==ch (classes 0-7)
            i2v = in2[:].rearrange("q (c d) -> q c d", d=D_DIM)
            nc.vector.tensor_copy(i2v[0:64, 0::2, :], v_sb[:, 0:8:2, :])
            nc.vector.tensor_copy(i2v[64:128, 1::2, :], v_sb[:, 1:8:2, :])
            # in2b cols 96:128 = classes 8,9 (rows 96:128 of vT4 after transpose)
            nc.vector.tensor_copy(in2b[0:64, 96:112], v_sb[:, 8, :])
            nc.vector.tensor_copy(in2b[64:128, 112:128], v_sb[:, 9, :])
            pv = psT.tile([128, I_CAPS], f32, tag="pt")
            nc.tensor.transpose(pv[:, 0:128], in2[:], eye_sb[:])
            nc.vector.tensor_copy(vT[:], pv[:, 0:128])
            pv4 = psT.tile([128, I_CAPS], f32, tag="pt")
            nc.tensor.transpose(pv4[:, 0:128], in2b[:], eye_sb[:])
            nc.scalar.copy(vT4[:], pv4[:, 0:128])

        def TA_phase(bts):
            for p in range(5):
                vrhs = vT[32 * p : 32 * (p + 1), :] if p < 4 else vT4[96:128, :]
                lhsW = WT if p < 4 else WTB
                row0 = 32 * p if p < 4 else 96
                Tp = itp.tile([128, K_DIM, NG, 128], bf16, tag="tp")
                for k in range(K_DIM):
                    pt = psT.tile([128, I_CAPS], f32, tag="pt")
                    for g in range(NG):
                        nc.tensor.matmul(
                            pt[:, 128 * g : 128 * (g + 1)],
                            lhsW[row0 : row0 + 32, k, 128 * g : 128 * (g + 1)],
                            vrhs,
                            start=True,
                            stop=True,
                            tile_position=(row0, 0),
                        )
                    nc.scalar.copy(
                        Tp[:, k, :, :].rearrange("r g q -> r (g q)"), pt[:]
                    )
                # P = T * u in k-halves (start after 4 evacs, not 8),
                # then tree-reduce over k
                P = itp.tile([128, K_DIM, NG, 128], bf16, tag="pp")
                for h in range(4):
                    k0 = 2 * h
                    nc.vector.tensor_tensor(
                        P[:, k0 : k0 + 2].rearrange(
                            "r k g (c b) -> r k g c b", c=2
                        ),
                        Tp[:, k0 : k0 + 2].rearrange(
                            "r k g (c b) -> r k g c b", c=2
                        ),
                        uTk[:, k0 : k0 + 2].rearrange(
                            "r k g b -> r k g () b"
                        ).to_broadcast((128, 2, NG, 2, B_PER)),
                        OP.mult,
                    )
                t1a = itp.tile([128, 4, NG, 128], bf16, tag="t4", bufs=1)
                nc.vector.tensor_tensor(t1a[:], P[:, 0:4], P[:, 4:8], OP.add)
                t2a = itp.tile([128, 2, NG, 128], bf16, tag="t2", bufs=1)
                nc.vector.tensor_tensor(t2a[:], t1a[:, 0:2], t1a[:, 2:4], OP.add)
                Lp = itp.tile([128, NG, 128], bf16, tag="t4", bufs=1)
                nc.vector.tensor_tensor(Lp[:], t2a[:, 0], t2a[:, 1], OP.add)
                Lv = L[:, p, :, :, :].rearrange("r g c b -> r (g c b)")
                nc.vector.tensor_tensor(
                    Lv, Lp[:].rearrange("r g q -> r (g q)"), Lv, OP.add
                )
                softmax_exp_p(p, bts)

        def softmax_exp_p(p, bts):
            """exp of pass p + ch-fold + progressive pair-sum during TA."""
            nc.scalar.activation(
                cE[:, p].rearrange("r g c b -> r (g c b)"),
                L[:, p].rearrange("r g c b -> r (g c b)"),
                AF.Exp,
            )
            nc.vector.tensor_tensor(
                bts[p][:], cE[:, p, :, 0, :], cE[:, p, :, 1, :], OP.add
            )
            if p == 1:
                nc.vector.tensor_tensor(bts[0][:], bts[0][:], bts[1][:], OP.add)
            elif p == 3:
                nc.vector.tensor_tensor(bts[2][:], bts[2][:], bts[3][:], OP.add)

        def softmax_phase(bts):
            b0, b1, b2, b3, b4 = bts
            den = smp.tile([128, NG, B_PER], f32, tag="smd", bufs=1)
            bf32 = smp.tile([128, NG, B_PER], f32, tag="smf", bufs=1)
            nc.vector.tensor_tensor(bf32[:], b0[:], b2[:], OP.add)
            nc.vector.tensor_tensor(den[:], bf32[:], b4[:], OP.add)
            with nc.allow_low_precision(reason="softmax reciprocal to bf16 ok"):
                nc.vector.reciprocal(
                    recT[:].rearrange("r g b -> r (g b)"),
                    den[:].rearrange("r g b -> r (g b)"),
                )

        def s_phase_routed():
            uTs = itp.tile([128, K_DIM, NG, B_PER], bf16, tag="uts", bufs=1)
            nc.vector.tensor_tensor(
                uTs[:],
                uTk[:],
                recT[:].rearrange("r g b -> r () g b").to_broadcast(
                    (128, K_DIM, NG, B_PER)
                ),
                OP.mult,
            )
            for c in range(C_CLS):
                p, ch = c // 2, c % 2
                xc = itp.tile([128, K_DIM, NG, B_PER], bf16, tag="pp")
                nc.vector.tensor_tensor(
                    xc[:],
                    uTs[:],
                    cE[:, p, :, ch, :].rearrange("r g b -> r () g b").to_broadcast(
                        (128, K_DIM, NG, B_PER)
                    ),
                    OP.mult,
                )
                ps = psS.tile([64, C_CLS * D_DIM], f32, tag="ps_s")
                n = 0
                for k in range(K_DIM):
                    for g in range(NG):
                        nc.tensor.matmul(
                            ps[:, 16 * c : 16 * (c + 1)],
                            xc[:, k, g, :],
                            Wsk[:, k, g, c, :],
                            start=(n == 0),
                            stop=(n == K_DIM * NG - 1),
                        )
                        n += 1
                nc.scalar.copy(s_sb[:, c, :], ps[:, 16 * c : 16 * (c + 1)])

        # ---------------- main flow ----------------
        import os
        kstage = int(os.environ.get("KSTAGE", "99"))
        s_phase_s0()
        squash()
        if kstage >= 1:
            for j in range(2):
                build_vT()
                bts = []
                for i in range(5):
                    bti = smp.tile(
                        [128, NG, B_PER], bf16, tag=f"sm{i}", bufs=1,
                        name=f"bt{i}",
                    )
                    bts.append(bti)
                TA_phase(bts)
                if kstage == 1 + 3 * j:
                    break
                softmax_phase(bts)
                if kstage == 2 + 3 * j:
                    break
                s_phase_routed()
                squash()
                if kstage == 3 + 3 * j:
                    break
        nc.sync.dma_start(v_out[:], v_sb[:])

        for pool in (smp, itp, psT, psS, perm):
            try:
                pool.release()
            except Exception:
                pass

    nc.compile()
    return nc


def _consts():
    return {"eye128": np.eye(128, dtype=np.float32)}


def _prep_w(W0):
    """Host-side layout marshalling of the replicated weights (pure
    permutation + bf16 cast; done once, shared by all cores)."""
    import ml_dtypes

    bf = ml_dtypes.bfloat16
    W0 = np.ascontiguousarray(W0, dtype=np.float32)  # [1152, 10, 16, 8]
    wsk = np.ascontiguousarray(
        W0.reshape(NG, 128, C_CLS, D_DIM, K_DIM).transpose(1, 4, 0, 2, 3)
    ).astype(bf)  # [128, k, g, c, d]
    wt = np.ascontiguousarray(
        W0[:, 0:8].transpose(1, 2, 3, 0).reshape(128, K_DIM, I_CAPS)
    ).astype(bf)  # rows 16c+d, classes 0-7
    wtb = np.ascontiguousarray(
        W0[:, 2:10].transpose(1, 2, 3, 0).reshape(128, K_DIM, I_CAPS)
    ).astype(bf)  # rows 16(c-2)+d; classes 8,9 at 96:128
    return wsk, wt, wtb


def _prep_u(ush):
    import ml_dtypes

    return np.ascontiguousarray(
        ush.reshape(B_PER, NG, 128, K_DIM).transpose(2, 3, 1, 0)
    ).astype(ml_dtypes.bfloat16)  # [128, k, g, b]


def get_nc():
    if "nc" not in _CACHE:
        _CACHE["nc"] = _build()
    return _CACHE["nc"]


def make_in_maps(u, W):
    consts = _consts()
    wsk, wt, wtb = _prep_w(W[0])
    in_maps = []
    for core in range(N_CORES):
        sh = np.ascontiguousarray(
            u[core * B_PER : (core + 1) * B_PER], dtype=np.float32
        )
        in_maps.append(
            {
                "uTk_h": _prep_u(sh),
                "wsk_h": wsk,
                "wt_h": wt,
                "wtb_h": wtb,
                **consts,
            }
        )
    return in_maps


def kernel(u: np.ndarray, W: np.ndarray) -> np.ndarray:
    from concourse.bass_utils import run_bass_kernel_spmd

    nc = get_nc()
    in_maps = make_in_maps(u, W)
    res = run_bass_kernel_spmd(nc, in_maps, list(range(N_CORES)))
    out = np.concatenate([res.results[i]["v"] for i in range(N_CORES)], axis=0)
    return out.astype(np.float32)



# revision 2
# speedup vs baseline: 1.0270x; 1.0270x over previous
"""DigitCaps (capsule routing) Trainium2 Bass kernel, v2.

u [512, 1152, 8] f32, W [1, 1152, 10, 16, 8] f32 -> v [512, 10, 16] f32
(3 dynamic-routing iterations, softmax over 10 classes).

Pure data-parallel: batch 64 per core x 8 cores; everything on-chip;
u_hat (377MB) never materialized. Per routing iteration:
  T[b,i,c,k] = sum_d W[i,c,d,k] v[b,c,d]     PE -> PSUM
  evac to bf16 (ACT) or fused mul (DVE-from-PSUM), P = T*u
  Linc[b,i,c] = sum_k P                      PE eye-matmul accumulate
  cE = exp(Linc) [* cE_prev]                 ACT exp from PSUM (+DVE mul)
  den folds on GPSIMD; recip DVE
  xc_c = (u*recT) * cE_c                     DVE / GPSIMD split
  s[b,c,:] = sum_{ik} W xc_c                 PE accumulating matmuls
  v = squash(s)
exp(L1+L2) == exp(L1)*exp(L2), so logits are never materialized.

Layouts (per core, B=64):
  i: block g = i//128 (9 blocks), partition r = i%128
  class c = 2p+ch, pass p in [0,5), parity ch in {0,1}
  exp/cE: [r, p, (g, ch, b)]
"""

import os
import numpy as np

N_CORES = 8
B_PER = 64
I_CAPS = 1152
K_DIM = 8
C_CLS = 10
D_DIM = 16
NG = I_CAPS // 128  # 9
EPS = 1e-8

# --- schedule knobs (cost-model balancing) ---
Z_KS = tuple(
    int(x) for x in os.environ.get("KV2_ZKS", "1,3,5").split(",") if x != ""
)  # k-indices whose T*u mul reads PSUM directly on DVE (no ACT evac)
POOL_MUL_KS = tuple(
    int(x) for x in os.environ.get("KV2_PMKS", "").split(",") if x != ""
)  # k-indices whose (evac'd) mul runs on GPSIMD
XC_POOL = tuple(
    int(x) for x in os.environ.get("KV2_XCPOOL", "").split(",") if x != ""
)  # classes whose xc mul runs on GPSIMD
XC_DMA = tuple(
    int(x) for x in os.environ.get("KV2_XCDMA", "").split(",") if x != ""
)  # classes whose xc mul runs as SP-copy + gpsimd DMA-accum-mult
XC_GPOOL = int(os.environ.get("KV2_XCGPOOL", "2"))  # trailing g-blocks on Pool
FOLDS_POOL = os.environ.get("KV2_FOLDSPOOL", "1") == "1"
CE_POOL = os.environ.get("KV2_CEPOOL", "1") == "1"

_CACHE = {}


def _build():
    import concourse.bass as bass
    import concourse.mybir as mybir
    from concourse import tile, bacc

    f32 = mybir.dt.float32
    bf16 = mybir.dt.bfloat16
    AF = mybir.ActivationFunctionType
    OP = mybir.AluOpType

    nc = bacc.Bacc()
    uTk_in = nc.dram_tensor(
        "uTk_h", [128, K_DIM, NG, B_PER], bf16, kind="ExternalInput"
    )
    wsk_in = nc.dram_tensor(
        "wsk_h", [128, K_DIM, NG, C_CLS, D_DIM], bf16, kind="ExternalInput"
    )
    # wt cols 0:1152 = rows 16c+d classes 0-7 (all 128 partitions);
    # cols 1152:2304 = rows 16(c-8)+d classes 8,9 (partitions 0:32)
    wt_in = nc.dram_tensor("wt_h", [128, K_DIM, 2 * I_CAPS], bf16, kind="ExternalInput")
    eyebf = nc.dram_tensor("eyebf", [128, 128], bf16, kind="ExternalInput")
    v_out = nc.dram_tensor("v", [B_PER, C_CLS, D_DIM], f32, kind="ExternalOutput")
    vdr = nc.dram_tensor("vdr", [2, 5, B_PER, 2, D_DIM], bf16, kind="Internal")

    with tile.TileContext(nc) as tc:
        perm = tc.alloc_tile_pool(name="perm", bufs=1)
        Wsk = perm.tile([128, K_DIM, NG, C_CLS, D_DIM], bf16)  # [r,(k,g,c,d)]
        WT = perm.tile([128, K_DIM, 2 * I_CAPS], bf16)
        uTk = perm.tile([128, K_DIM, NG, B_PER], bf16)      # u[b, 128g+r, k]
        # exp(L); layout [r, p, ch, g, b] so per-(p,ch) slices are contiguous
        cEa = perm.tile([128, 5, 2, NG, B_PER], bf16, name="cEa")
        cEb = perm.tile([128, 5, 2, NG, B_PER], bf16, name="cEb")
        recT = perm.tile([128, NG, B_PER], bf16, name="recTt")    # 1/den i-major
        vT = perm.tile([128, 128], bf16)             # block-diag v^T classes 0-7
        vT4 = perm.tile([128, 128], bf16)            # rows 0:32: classes 8,9
        v_sb = perm.tile([64, C_CLS, D_DIM], f32, name="vsbt")
        vbf = perm.tile([64, C_CLS, D_DIM], bf16, name="vbft")
        s_sb = perm.tile([64, C_CLS, D_DIM], f32, name="ssbt")
        eyeb_sb = perm.tile([128, 128], bf16)
        sq = perm.tile([64, C_CLS, D_DIM], f32)
        n2 = perm.tile([64, C_CLS], f32)
        t1 = perm.tile([64, C_CLS], f32)
        r1 = perm.tile([64, C_CLS], f32)
        f1 = perm.tile([64, C_CLS], f32)
        nrm = perm.tile([64, C_CLS], f32)
        nrm2 = perm.tile([64, C_CLS], f32)
        r2 = perm.tile([64, C_CLS], f32)
        fac = perm.tile([64, C_CLS], f32)
        den = perm.tile([128, NG, B_PER], f32, name="dent")
        dtmp = perm.tile([128, NG, B_PER], f32, name="dtmpt")

        psT = tc.alloc_tile_pool(name="psT", bufs=2, space="PSUM")
        psL = tc.alloc_tile_pool(name="psL", bufs=2, space="PSUM")

        # ---------------- setup: inputs arrive pre-arranged ----
        nc.sync.dma_start(uTk[:, 0:4], uTk_in[:, 0:4])
        nc.sync.dma_start(Wsk[:, 0:4], wsk_in[:, 0:4])
        nc.sync.dma_start(uTk[:, 4:8], uTk_in[:, 4:8])
        nc.sync.dma_start(Wsk[:, 4:8], wsk_in[:, 4:8])
        nc.sync.dma_start(eyeb_sb[:], eyebf[:])
        nc.sync.dma_start(WT[:, 0:4, 0:I_CAPS], wt_in[:, 0:4, 0:I_CAPS])
        nc.sync.dma_start(WT[0:32, 0:4, I_CAPS:], wt_in[0:32, 0:4, I_CAPS:])
        nc.sync.dma_start(WT[:, 4:8, 0:I_CAPS], wt_in[:, 4:8, 0:I_CAPS])
        nc.sync.dma_start(WT[0:32, 4:8, I_CAPS:], wt_in[0:32, 4:8, I_CAPS:])

        nc.gpsimd.memset(vT[:], 0.0)
        nc.gpsimd.memset(vT4[:], 0.0)

        itp = tc.alloc_tile_pool(name="itp", bufs=2)
        smp = tc.alloc_tile_pool(name="smp", bufs=3)

        def s_phase_s0():
            pst = psL.tile([128, 512], f32, tag="lp")
            ps = pst[0:64, 0 : C_CLS * D_DIM]
            n = 0
            for k in range(K_DIM):
                for g in range(NG):
                    nc.tensor.matmul(
                        ps,
                        uTk[:, k, g, :],
                        Wsk[:, k, g, :, :].rearrange("r c d -> r (c d)"),
                        start=(n == 0),
                        stop=(n == K_DIM * NG - 1),
                    )
                    n += 1
            nc.scalar.activation(
                s_sb[:].rearrange("b c d -> b (c d)"), ps, AF.Copy, scale=0.1
            )

        def squash():
            nc.scalar.square(sq[:], s_sb[:])
            nc.vector.reduce_sum(n2[:], sq[:], axis=mybir.AxisListType.X)
            nc.scalar.add(t1[:], n2[:], 1.0)
            nc.vector.reciprocal(r1[:], t1[:])
            nc.vector.tensor_mul(f1[:], n2[:], r1[:])
            nc.scalar.sqrt(nrm[:], n2[:])
            nc.vector.tensor_scalar_add(nrm2[:], nrm[:], EPS)
            nc.vector.reciprocal(r2[:], nrm2[:])
            nc.vector.tensor_mul(fac[:], f1[:], r2[:])
            for c in range(C_CLS):
                nc.vector.tensor_scalar_mul(
                    v_sb[:, c, :], s_sb[:, c, :], fac[:, c : c + 1]
                )

        def squash_pair(p):
            """squash for classes 2p, 2p+1 only; writes bf16 vbf slices.
            fac = n2 / ((1 + n2) * (sqrt(n2) + EPS)), v = fac * s."""
            sqp = smp.tile([64, 2, D_DIM], f32, tag="sqp", bufs=2)
            n2p = smp.tile([64, 2], f32, tag="n2p", bufs=2)
            nrp = smp.tile([64, 2], f32, tag="nrp", bufs=2)
            dnp = smp.tile([64, 2], f32, tag="dnp", bufs=2)
            rcp = smp.tile([64, 2], f32, tag="rcp", bufs=2)
            fcp = smp.tile([64, 2], f32, tag="fcp", bufs=2)
            sv = s_sb[:, 2 * p : 2 * p + 2, :]
            nc.scalar.square(sqp[:], sv)
            nc.vector.reduce_sum(n2p[:], sqp[:], axis=mybir.AxisListType.X)
            nc.scalar.sqrt(nrp[:], n2p[:])
            nc.vector.tensor_scalar_add(nrp[:], nrp[:], EPS)
            # dnp = (n2p + 1) * nrp
            nc.vector.scalar_tensor_tensor(
                dnp[:], n2p[:], 1.0, nrp[:], OP.add, OP.mult
            )
            nc.vector.reciprocal(rcp[:], dnp[:])
            nc.vector.tensor_mul(fcp[:], n2p[:], rcp[:])
            with nc.allow_low_precision(reason="v to bf16 for T matmuls"):
                for cc in range(2):
                    c = 2 * p + cc
                    nc.vector.tensor_scalar_mul(
                        vbf[:, c, :], sv[:, cc, :], fcp[:, cc : cc + 1]
                    )

        def vT_dma_pair(p, slot):
            """Bounce classes 2p,2p+1 of vbf through DRAM to transpose into
            the block-diagonal vT slots: vT[32p+16cc+d, 64cc+b] = v[b,2p+cc,d].
            All on SP/DMA; no compute engine involved."""
            nc.sync.dma_start(vdr[slot, p], vbf[:, 2 * p : 2 * p + 2, :])
            dst_tile = vT if p < 4 else vT4
            r0 = 32 * p if p < 4 else 0
            for cc in range(2):
                nc.sync.dma_start(
                    dst_tile[r0 + 16 * cc : r0 + 16 * cc + 16,
                             64 * cc : 64 * cc + 64],
                    vdr[slot, p, :, cc, :].rearrange("b d -> d b"),
                )

        def build_vT():
            with nc.allow_low_precision(reason="v to bf16 for T matmuls"):
                nc.vector.tensor_copy(vbf[:], v_sb[:])
            for p in range(5):
                vT_dma_pair(p, 0)

        def L_front(j, p):
            """T matmuls + evac/mul for pass p; returns the P tile."""
            if p < 4:
                vrhs, row0, col0 = vT[32 * p : 32 * (p + 1), :], 32 * p, 0
            else:
                vrhs, row0, col0 = vT4[0:32, :], 0, I_CAPS
            P = itp.tile([128, K_DIM, NG, 128], bf16, tag="pp")
            for k in range(K_DIM):
                pt = psT.tile([128, I_CAPS], f32, tag="pt")
                for g in range(NG):
                    nc.tensor.matmul(
                        pt[:, 128 * g : 128 * (g + 1)],
                        WT[row0 : row0 + 32, k, col0 + 128 * g : col0 + 128 * (g + 1)],
                        vrhs,
                        start=True,
                        stop=True,
                        tile_position=(row0, 0),
                    )
                ubc = uTk[:, k].rearrange("r g b -> r g () b").to_broadcast(
                    (128, NG, 2, B_PER)
                )
                pk = P[:, k].rearrange("r g (c b) -> r g c b", c=2)
                if k in Z_KS:
                    # fused: P_k = T_k(PSUM) * u_k on DVE
                    nc.vector.tensor_tensor(
                        pk,
                        pt[:].rearrange("r (g c b) -> r g c b", g=NG, c=2),
                        ubc,
                        OP.mult,
                    )
                else:
                    Tp = itp.tile([128, NG, 128], bf16, tag="tp", bufs=4)
                    nc.scalar.copy(Tp[:].rearrange("r g q -> r (g q)"), pt[:])
                    tv = Tp[:].rearrange("r g (c b) -> r g c b", c=2)
                    if k in POOL_MUL_KS:
                        nc.gpsimd.tensor_tensor(pk, tv, ubc, OP.mult)
                    else:
                        nc.vector.tensor_tensor(pk, tv, ubc, OP.mult)
            return P

        def L_back(j, p, P, bts, cE_prev, cE):
            """ksum + exp + cE product + denominator folds for pass p."""
            Pf = P[:].rearrange("r k g q -> r k (g q)")

            def cegv(t, g0, g1):
                # [r, g, ch, b] view over g block (enumeration order of Linc)
                return t[:, p, :, g0:g1, :].rearrange("r c g b -> r g c b")

            for g0, g1 in ((0, 4), (4, 8), (8, NG)):
                s, e = 128 * g0, 128 * g1
                lp = psL.tile([128, 512], f32, tag="lp")
                for k in range(K_DIM):
                    nc.tensor.matmul(
                        lp[:, 0 : e - s],
                        eyeb_sb[:],
                        Pf[:, k, s:e],
                        start=(k == 0),
                        stop=(k == K_DIM - 1),
                    )
                lpv = lp[:, 0 : e - s].rearrange(
                    "r (g c b) -> r g c b", c=2, b=B_PER
                )
                if j == 0:
                    nc.scalar.activation(cegv(cE, g0, g1), lpv, AF.Exp)
                else:
                    Er = itp.tile([128, 512], bf16, tag="er", bufs=3)
                    erv = Er[:, 0 : e - s].rearrange(
                        "r (g c b) -> r g c b", c=2, b=B_PER
                    )
                    nc.scalar.activation(erv, lpv, AF.Exp)
                    if CE_POOL:
                        nc.gpsimd.tensor_tensor(
                            cegv(cE, g0, g1), erv, cegv(cE_prev, g0, g1), OP.mult
                        )
                    else:
                        nc.vector.tensor_tensor(
                            cegv(cE, g0, g1), erv, cegv(cE_prev, g0, g1), OP.mult
                        )
            # progressive denominator folds
            eng = nc.gpsimd if FOLDS_POOL else nc.vector
            eng.tensor_tensor(
                bts[p][:], cE[:, p, 0], cE[:, p, 1], OP.add
            )
            if p == 1:
                eng.tensor_tensor(bts[0][:], bts[0][:], bts[1][:], OP.add)
            elif p == 3:
                eng.tensor_tensor(bts[2][:], bts[2][:], bts[3][:], OP.add)

        def L_phase(j, bts):
            """Software-pipelined: pass p's back-end is emitted after pass
            p+1's front-end so its cross-engine waits are already satisfied
            when they reach the strict-FIFO engine queues."""
            cE_prev, cE = (None, cEa) if j == 0 else (cEa, cEb)
            Ps = [None] * 5
            for p in range(5):
                Ps[p] = L_front(j, p)
                if p >= 1:
                    L_back(j, p - 1, Ps[p - 1], bts, cE_prev, cE)
            L_back(j, 4, Ps[4], bts, cE_prev, cE)
            return cE

        def softmax_phase(bts):
            nc.vector.tensor_tensor(dtmp[:], bts[0][:], bts[2][:], OP.add)
            nc.vector.tensor_tensor(den[:], dtmp[:], bts[4][:], OP.add)
            with nc.allow_low_precision(reason="softmax reciprocal to bf16 ok"):
                nc.vector.reciprocal(
                    recT[:].rearrange("r g b -> r (g b)"),
                    den[:].rearrange("r g b -> r (g b)"),
                )

        def s_phase_routed(cE, pipe_vT):
            uTs = itp.tile([128, K_DIM, NG, B_PER], bf16, tag="uts", bufs=1)
            nc.vector.tensor_tensor(
                uTs[:],
                uTk[:],
                recT[:].rearrange("r g b -> r () g b").to_broadcast(
                    (128, K_DIM, NG, B_PER)
                ),
                OP.mult,
            )
            def xc_front(c):
                p, ch = c // 2, c % 2
                xc = itp.tile([128, K_DIM, NG, B_PER], bf16, tag="xc")
                cbc = cE[:, p, ch].rearrange("r g b -> r () g b").to_broadcast(
                    (128, K_DIM, NG, B_PER)
                )
                if c in XC_DMA:
                    # xc = us (SP DMA copy), then xc *= cE_c (gpsimd DMA
                    # with CCE multiply; src broadcast over k)
                    nc.sync.dma_start(xc[:], uTs[:])
                    nc.gpsimd.dma_start(xc[:], cbc, accum_op=OP.mult)
                elif c in XC_POOL:
                    nc.gpsimd.tensor_tensor(xc[:], uTs[:], cbc, OP.mult)
                elif XC_GPOOL > 0:
                    gs = NG - XC_GPOOL
                    cb = cE[:, p, ch].rearrange("r g b -> r () g b")
                    nc.vector.tensor_tensor(
                        xc[:, :, 0:gs],
                        uTs[:, :, 0:gs],
                        cb[:, :, 0:gs].to_broadcast((128, K_DIM, gs, B_PER)),
                        OP.mult,
                    )
                    nc.gpsimd.tensor_tensor(
                        xc[:, :, gs:],
                        uTs[:, :, gs:],
                        cb[:, :, gs:].to_broadcast(
                            (128, K_DIM, XC_GPOOL, B_PER)
                        ),
                        OP.mult,
                    )
                else:
                    nc.vector.tensor_tensor(xc[:], uTs[:], cbc, OP.mult)
                return xc

            def s_back(c, xc):
                pst = psL.tile([128, 512], f32, tag="lp")
                ps = pst[0:64, 0:D_DIM]
                n = 0
                for k in range(K_DIM):
                    for g in range(NG):
                        nc.tensor.matmul(
                            ps,
                            xc[:, k, g, :],
                            Wsk[:, k, g, c, :],
                            start=(n == 0),
                            stop=(n == K_DIM * NG - 1),
                        )
                        n += 1
                nc.scalar.copy(s_sb[:, c, :], ps)

            def pair_done(cdone):
                """Classes 2p,2p+1 are in s_sb: squash the pair and DMA its
                block into vT so the next round's T matmuls can start."""
                if pipe_vT and cdone % 2 == 1:
                    p = cdone // 2
                    squash_pair(p)
                    vT_dma_pair(p, 1)

            xcs = [None] * C_CLS
            for c in range(C_CLS):
                xcs[c] = xc_front(c)
                if c >= 1:
                    s_back(c - 1, xcs[c - 1])
                    pair_done(c - 1)
            s_back(C_CLS - 1, xcs[C_CLS - 1])
            pair_done(C_CLS - 1)

        # ---------------- main flow ----------------
        kstage = int(os.environ.get("KSTAGE", "99"))
        s_phase_s0()
        squash()
        build_vT()
        if kstage >= 1:
            for j in range(2):
                bts = []
                for i in range(5):
                    bti = smp.tile(
                        [128, NG, B_PER], bf16, tag=f"sm{i}", bufs=2,
                        name=f"bt{i}_{j}",
                    )
                    bts.append(bti)
                cE = L_phase(j, bts)
                if kstage == 1 + 3 * j:
                    break
                softmax_phase(bts)
                if kstage == 2 + 3 * j:
                    break
                s_phase_routed(cE, pipe_vT=(j == 0))
                if j == 1:
                    squash()
                if kstage == 3 + 3 * j:
                    break
        nc.sync.dma_start(v_out[:], v_sb[:])

        for pool in (smp, itp, psL, psT, perm):
            try:
                pool.release()
            except Exception:
                pass

    nc.compile()
    return nc


def _consts():
    import ml_dtypes

    return {"eyebf": np.eye(128, dtype=np.float32).astype(ml_dtypes.bfloat16)}


def _prep_w(W0):
    """Host-side layout marshalling of the replicated weights (pure
    permutation + bf16 cast; done once, shared by all cores)."""
    import ml_dtypes

    bf = ml_dtypes.bfloat16
    W0 = np.ascontiguousarray(W0, dtype=np.float32)  # [1152, 10, 16, 8]
    wsk = np.ascontiguousarray(
        W0.reshape(NG, 128, C_CLS, D_DIM, K_DIM).transpose(1, 4, 0, 2, 3)
    ).astype(bf)  # [128, k, g, c, d]
    wt = np.zeros((128, K_DIM, 2 * I_CAPS), dtype=bf)
    wt[:, :, 0:I_CAPS] = (
        W0[:, 0:8].transpose(1, 2, 3, 0).reshape(128, K_DIM, I_CAPS).astype(bf)
    )  # rows 16c+d, classes 0-7
    wt[0:32, :, I_CAPS:] = (
        W0[:, 8:10].transpose(1, 2, 3, 0).reshape(32, K_DIM, I_CAPS).astype(bf)
    )  # rows 16(c-8)+d, classes 8,9
    return wsk, wt


def _prep_u(ush):
    import ml_dtypes

    return np.ascontiguousarray(
        ush.reshape(B_PER, NG, 128, K_DIM).transpose(2, 3, 1, 0)
    ).astype(ml_dtypes.bfloat16)  # [128, k, g, b]


def get_nc():
    if "nc" not in _CACHE:
        _CACHE["nc"] = _build()
    return _CACHE["nc"]


def make_in_maps(u, W):
    consts = _consts()
    wsk, wt = _prep_w(W[0])
    in_maps = []
    for core in range(N_CORES):
        sh = np.ascontiguousarray(
            u[core * B_PER : (core + 1) * B_PER], dtype=np.float32
        )
        in_maps.append(
            {
                "uTk_h": _prep_u(sh),
                "wsk_h": wsk,
                "wt_h": wt,
                **consts,
            }
        )
    return in_maps


def kernel(u: np.ndarray, W: np.ndarray) -> np.ndarray:
    from concourse.bass_utils import run_bass_kernel_spmd

    nc = get_nc()
    in_maps = make_in_maps(u, W)
    res = run_bass_kernel_spmd(nc, in_maps, list(range(N_CORES)))
    out = np.concatenate([res.results[i]["v"] for i in range(N_CORES)], axis=0)
    return out.astype(np.float32)
